# revision 16
# baseline (speedup 1.0000x reference)
"""Fake-quantized multi-head attention block on 8 TRN2 NeuronCores.

Data-parallel over batch (1 element per core); integer-domain quantized
matmuls in bf16; global fake-quant scales via tiny AllReduce(max)
collectives. Engine-balanced v2: rowmax tree on gpsimd, split AR#2,
persistent padded-V tiles, [0:66]-sliced aug operands.
"""

import sys

sys.path.insert(0, "/opt/trn_rl_repo")

import numpy as np
import ml_dtypes

import concourse.mybir as mybir
import concourse.tile as tile
import concourse.bass_isa as bass_isa
from concourse import bacc
from concourse.bass_utils import run_bass_kernel_spmd

f32 = mybir.dt.float32
bf16 = mybir.dt.bfloat16
ALU = mybir.AluOpType
ACT = mybir.ActivationFunctionType
AX = mybir.AxisListType
RED = bass_isa.ReduceOp

B, N, C = 8, 1024, 768
H, HD = 12, 64
NCORES = 8
MAGIC = float(np.float32(3 * 2**22))
CORES = list(range(NCORES))
RG = [CORES]

SFIN_CONST = [1.0]
CATT_CONST = [1.0]


def _warm_burst(nc, pool, sbpool, wut, anchor, n, tagix, tag, shape, bufs=None):
    """Keep-warm dummy matmuls anchored to a late-ready tile so they run
    exactly during a collective's latency window (PE otherwise idle)."""
    anch = sbpool.tile([128, 512], bf16, tag=f"anc{tagix}", name=f"anc{tagix}")
    nc.vector.memset(anch[:], 1.0)
    w = min(512, anchor.shape[-1])
    nc.vector.tensor_copy(anch[:, 0:w], anchor[:, 0:w])
    wps = pool.tile(shape, f32, tag=tag, name=f"wub{tagix}", bufs=bufs)
    wps2 = pool.tile(shape, f32, tag=tag, name=f"wub{tagix}b", bufs=bufs)
    for i in range(n):
        nc.tensor.matmul(
            (wps if i % 2 == 0 else wps2)[:, 0:512],
            lhsT=wut[:, 0:128], rhs=anch[:], start=True, stop=True,
        )


def build_graph(inv_s_x: float):
    nc = bacc.Bacc("TRN2", target_bir_lowering=False, debug=False, num_devices=NCORES)

    xT_ext = nc.dram_tensor("xT", [C, N], f32, kind="ExternalInput")
    wq_qkv_ext = nc.dram_tensor("wq_qkv", [C, 3 * C], bf16, kind="ExternalInput")
    wq_proj_ext = nc.dram_tensor("wq_proj", [C, C], bf16, kind="ExternalInput")
    bqs_ext = nc.dram_tensor("bqs", [1, 3 * C], f32, kind="ExternalInput")
    bp_ext = nc.dram_tensor("bp", [1, C], f32, kind="ExternalInput")
    out_ext = nc.dram_tensor("out", [N, C], f32, kind="ExternalOutput")

    with tile.TileContext(nc) as tc:
        run_body(nc, tc, inv_s_x, xT_ext, wq_qkv_ext, wq_proj_ext, bqs_ext, bp_ext, out_ext)
    nc.finalize()
    return nc


def run_body(nc, tc, inv_s_x, xT_ext, wq_qkv_ext, wq_proj_ext, bqs_ext, bp_ext, out_ext):
    with (
        tc.tile_pool(name="persist", bufs=1) as pp,
        tc.tile_pool(name="dram", bufs=1, space="DRAM") as dram,
    ):
        # aug tensors: rows 0:64 = q/k head slices (ints); aug_k rows 64:66 =
        # ones; aug_q rows 64:66 = ln(r)/c hi/lo (zero during phase A, written
        # after AR#2b). All attention matmuls slice partitions [0:66], so rows
        # 66:128 are never initialized.
        aug_q = [pp.tile([128, N], bf16, tag=f"augq{h}", name=f"augq{h}") for h in range(H)]
        aug_k = [pp.tile([128, N], bf16, tag=f"augk{h}", name=f"augk{h}") for h in range(H)]
        zbuf = pp.tile([128, 96], f32, tag="zbuf", name="zbuf")
        mlbuf = pp.tile([128, 96], f32, tag="mlbuf", name="mlbuf")  # rowmax of ea
        qkv_abs = pp.tile([128, 24], f32, tag="qkv_abs", name="qkv_abs")
        v_abs = pp.tile([128, 16], f32, tag="v_abs", name="v_abs")
        o_abs = pp.tile([128, 6], f32, tag="o_abs", name="o_abs")
        sc = pp.tile([128, 16], f32, tag="sc", name="sc")
        bqs_cols = pp.tile([128, 12], f32, tag="bqs_cols", name="bqs_cols")
        bv_bc = pp.tile([128, C], f32, tag="bv_bc", name="bv_bc")
        magic_col = pp.tile([128, 1], f32, tag="magic_col", name="magic_col")

        ar1_in = dram.tile([1, 2], f32, tag="ar1_in", name="ar1_in")
        ar1_out = dram.tile([1, 2], f32, tag="ar1_out", name="ar1_out")
        ar2a_in = dram.tile([1, 1], f32, tag="ar2a_in", name="ar2a_in")
        ar2a_out = dram.tile([1, 1], f32, tag="ar2a_out", name="ar2a_out")
        ar2b_in = dram.tile([1, 1], f32, tag="ar2b_in", name="ar2b_in")
        ar2b_out = dram.tile([1, 1], f32, tag="ar2b_out", name="ar2b_out")
        ar3_in = dram.tile([1, 1], f32, tag="ar3_in", name="ar3_in")
        ar3_out = dram.tile([1, 1], f32, tag="ar3_out", name="ar3_out")

        # PE warm-up burst first (flip HAM to 8/8)
        wut = pp.tile([128, 512], bf16, tag="wut", name="wut")
        nc.vector.memset(wut[:], 1.0)
        with tc.tile_pool(name="pswu", bufs=1, space="PSUM") as pswu:
            wps = pswu.tile([128, 512], f32, tag="wps", name="wps")
            for _ in range(20):
                nc.tensor.matmul(wps[:], lhsT=wut[:, 0:128], rhs=wut[:], start=True, stop=True)
        nc.vector.memset(magic_col[:], MAGIC)
        for h in range(H):
            nc.vector.memset(aug_q[h][64:66, :], 0.0)
            nc.vector.memset(aug_k[h][64:66, :], 1.0)
        with tc.tile_pool(name="brow", bufs=1) as br:
            bvrow = br.tile([1, C], f32, tag="bvrow", name="bvrow")
            nc.sync.dma_start(bvrow[:], bqs_ext[0:1, 1536:2304])
            nc.gpsimd.partition_broadcast(bv_bc[:], bvrow[:])

        # late pool: tensors born mid-kernel (vz at v-quant, bp_bc at proj)
        with tc.tile_pool(name="late", bufs=1) as lp:
         # padded PV weights, persistent: vz[hp][:, jt*256 + 0:64] = v cols of
         # head 2hp (key-block jt), [jt*256+192:256] = head 2hp+1, gaps zero.
         vz = [lp.tile([128, 8 * 256], bf16, tag=f"vz{t}", name=f"vz{t}") for t in range(6)]
         bp_bc = lp.tile([128, C], f32, tag="bp_bc", name="bp_bc")
         # v_f persists until v-quantize (after AR#2a)
         with tc.tile_pool(name="vf_pool", bufs=1) as vfp:
          v_f = [vfp.tile([128, C], f32, tag=f"vf{t}", name=f"vf{t}") for t in range(8)]

          # ---- stage 1+2: x quant, QKV matmuls, AR#1 (q,k), quantize -----
          with (
            tc.tile_pool(name="wload", bufs=1) as wl,
            tc.tile_pool(name="qkvf_pool", bufs=1) as qp,
            tc.tile_pool(name="s12", bufs=2) as s12,
            tc.tile_pool(name="psq", bufs=4, space="PSUM") as psq,
          ):
            wq_bf = [wl.tile([128, 3 * C], bf16, tag=f"wq{t}", name=f"wq{t}") for t in range(6)]
            xq = [wl.tile([128, N], bf16, tag=f"xq{t}", name=f"xq{t}") for t in range(6)]
            qkv_f = [qp.tile([128, N], f32, tag=f"qkvf{t}", name=f"qkvf{t}") for t in range(12)]

            for t in range(6):
                xf = s12.tile([128, N], f32, tag="s12y", name="xf")
                nc.sync.dma_start(xf[:], xT_ext[t * 128 : (t + 1) * 128, :])
                nc.sync.dma_start(wq_bf[t][:], wq_qkv_ext[t * 128 : (t + 1) * 128, :])
                # x quantize: scale+magic on Act (idle in this window), round on DVE
                y = s12.tile([128, N], f32, tag="s12y", name="y")
                nc.scalar.activation(y[:], xf[:], ACT.Identity, bias=magic_col[:], scale=inv_s_x)
                nc.vector.tensor_scalar(xq[t][:], y[:], MAGIC, None, ALU.subtract)
            for mt in range(12):
                nc.sync.dma_start(
                    bqs_cols[:, mt : mt + 1],
                    bqs_ext[0:1, mt * 128 : (mt + 1) * 128].rearrange("a (p b) -> (a p) b", b=1),
                )

            # q/k part: per-mt accumulation over kt
            for mt in range(12):
                pss = [psq.tile([128, 512], f32, tag="psq", name="psq", bufs=6) for _ in range(2)]
                for kt in range(6):
                    for nk in range(2):
                        mi = nc.tensor.matmul(
                            pss[nk][:],
                            lhsT=wq_bf[kt][:, mt * 128 : (mt + 1) * 128],
                            rhs=xq[kt][:, nk * 512 : (nk + 1) * 512],
                            start=(kt == 0),
                            stop=(kt == 5),
                            skip_group_check=True,
                        )
                        if nk == 1:
                            mi.ins.ldweights = False
                for nk in range(2):
                    # bias-add + PSUM->SBUF on Act; absmax post-bias on DVE
                    nc.scalar.activation(
                        qkv_f[mt][:, nk * 512 : (nk + 1) * 512],
                        pss[nk][:],
                        ACT.Identity,
                        bias=bqs_cols[:, mt : mt + 1],
                    )
                    nc.vector.tensor_reduce(
                        qkv_abs[:, mt * 2 + nk : mt * 2 + nk + 1],
                        qkv_f[mt][:, nk * 512 : (nk + 1) * 512],
                        axis=AX.X,
                        op=ALU.max,
                        apply_absolute_value=True,
                    )

            # ---- AR#1: global absmax of q, k only --------------------------
            # (absmax computed pre-bias on PSUM would be wrong; bias shifts
            # values. Reduce from qkv_f instead -- see note below.)
            am2 = s12.tile([128, 2], f32, tag="am2", name="am2")
            nc.vector.tensor_reduce(am2[:, 0:1], qkv_abs[:, 0:12], axis=AX.X, op=ALU.max)
            nc.vector.tensor_reduce(am2[:, 1:2], qkv_abs[:, 12:24], axis=AX.X, op=ALU.max)
            am2r = s12.tile([128, 2], f32, tag="am2r", name="am2r")
            nc.gpsimd.partition_all_reduce(am2r[:], am2[:], 128, RED.max)
            nc.gpsimd.dma_start(ar1_in[:], am2r[0:1, :])
            nc.gpsimd.collective_compute(
                "AllReduce", ALU.max, replica_groups=RG, ins=[ar1_in.opt()], outs=[ar1_out.opt()]
            )
            # keep-warm during AR#1 (runs alongside the v matmuls below)
            _warm_burst(nc, psq, s12, wut, am2r, 20, 1, tag="psq", shape=[128, 512], bufs=6)
            g2 = pp.tile([1, 2], f32, tag="g2", name="g2")
            nc.gpsimd.dma_start(g2[:], ar1_out[:])
            g2b = pp.tile([128, 2], f32, tag="g2b", name="g2b")
            nc.gpsimd.partition_broadcast(g2b[:], g2[:])

            inv2 = pp.tile([128, 2], f32, tag="inv2", name="inv2")
            nc.vector.reciprocal(inv2[:], g2b[:])
            nc.vector.tensor_scalar(inv2[:], inv2[:], 127.0, None, ALU.mult)
            nc.vector.tensor_tensor(sc[:, 3:4], g2b[:, 0:1], g2b[:, 1:2], ALU.mult)
            nc.vector.tensor_scalar(sc[:, 3:4], sc[:, 3:4], CATT_CONST[0], None, ALU.mult)
            nc.vector.reciprocal(sc[:, 9:10], sc[:, 3:4])
            # v matmuls fill the PE idle window during/after AR#1
            for nt in range(8):
                pss = []
                for ick, (ck, cw) in enumerate(((0, 512), (512, 256))):
                    pss.append((psq.tile([128, 512], f32, tag="psq", name="psv", bufs=6), ck, cw))
                for kt in range(6):
                    for ick2, (ps, ck, cw) in enumerate(pss):
                        mi = nc.tensor.matmul(
                            ps[:, 0:cw],
                            lhsT=xq[kt][:, nt * 128 : (nt + 1) * 128],
                            rhs=wq_bf[kt][:, 1536 + ck : 1536 + ck + cw],
                            start=(kt == 0),
                            stop=(kt == 5),
                            skip_group_check=True,
                        )
                        if ick2 == 1:
                            mi.ins.ldweights = False
                for ick, (ps, ck, cw) in enumerate(pss):
                    nc.vector.scalar_tensor_tensor(
                        v_f[nt][:, ck : ck + cw],
                        ps[:, 0:cw],
                        1.0,
                        bv_bc[:, ck : ck + cw],
                        ALU.mult,
                        ALU.add,
                    )
                    nc.vector.tensor_reduce(
                        v_abs[:, nt * 2 + ick : nt * 2 + ick + 1],
                        v_f[nt][:, ck : ck + cw],
                        axis=AX.X,
                        op=ALU.max,
                        apply_absolute_value=True,
                    )

            # ---- AR#2a: global absmax of v, issued EARLY (hides under
            # phase A; result needed only at v-quantize) -------------------
            vam = s12.tile([128, 1], f32, tag="vam", name="vam")
            nc.vector.tensor_reduce(vam[:], v_abs[:], axis=AX.X, op=ALU.max)
            vamr = s12.tile([128, 1], f32, tag="vamr", name="vamr")
            nc.gpsimd.partition_all_reduce(vamr[:], vam[:], 128, RED.max)
            nc.gpsimd.dma_start(ar2a_in[:], vamr[0:1, :])
            nc.gpsimd.collective_compute(
                "AllReduce", ALU.max, replica_groups=RG, ins=[ar2a_in.opt()], outs=[ar2a_out.opt()]
            )
            gmv = pp.tile([1, 1], f32, tag="gmv", name="gmv")
            nc.gpsimd.dma_start(gmv[:], ar2a_out[:])
            nc.gpsimd.partition_broadcast(sc[:, 14:15], gmv[:])
            nc.vector.reciprocal(sc[:, 15:16], sc[:, 14:15])
            nc.vector.tensor_scalar(sc[:, 15:16], sc[:, 15:16], 127.0, None, ALU.mult)

            # ---- quantize q/k; build aug tensors ---------------------------
            # op1 (scale+magic, AP scalar) on DVE; op2 (round, imm) on gpsimd
            for mt in (0, 6, 1, 7, 2, 8, 3, 9, 4, 10, 5, 11):
                inv = inv2[:, 0:1] if mt < 6 else inv2[:, 1:2]
                y = s12.tile([128, N], f32, tag="s12y", name="yq")
                nc.vector.tensor_scalar(y[:], qkv_f[mt][:], inv, MAGIC, ALU.mult, ALU.add)
                qsc = s12.tile([128, N], bf16, tag="qsc", name="qsc")
                nc.gpsimd.tensor_scalar(qsc[:], y[:], MAGIC, None, ALU.subtract)
                dst = aug_q if mt < 6 else aug_k
                tt = mt if mt < 6 else mt - 6
                nc.sync.dma_start(dst[2 * tt][0:64, :], qsc[0:64, :])
                nc.sync.dma_start(dst[2 * tt + 1][0:64, :], qsc[64:128, :])

            # zero-fill vz gaps on gpsimd (idle until phase B; must precede
            # the strided v-round writes at AR#2b)
            for t in range(6):
                nc.gpsimd.memset(vz[t][:], 0.0)

          # ---- phase A: attn logits stats (z via Act accum, rowmax of ea
          # via gpsimd max-tree + small DVE reduce) ------------------------
          with (
            tc.tile_pool(name="phA", bufs=4) as pa,
            tc.tile_pool(name="psA", bufs=3, space="PSUM") as psa,
          ):
            for h in range(H):
                for itp in range(4):
                    # paired it-blocks: one [128, 2N] ea tile, one DVE rowmax
                    ea = pa.tile([128, 2 * N], bf16, tag="ea", name="ea")
                    for s in range(2):
                        it = 2 * itp + s
                        psl = psa.tile([128, N], f32, tag="psl", name="psl")
                        for jc in range(2):
                            mi = nc.tensor.matmul(
                                psl[:, jc * 512 : (jc + 1) * 512],
                                lhsT=aug_q[h][0:66, it * 128 : (it + 1) * 128],
                                rhs=aug_k[h][0:66, jc * 512 : (jc + 1) * 512],
                                start=True,
                                stop=True,
                            )
                            if jc == 1:
                                mi.ins.ldweights = False
                        col = h * 8 + it
                        nc.scalar.activation(
                            ea[:, s * N : (s + 1) * N], psl[:], ACT.Exp, scale=sc[:, 3:4],
                            accum_out=zbuf[:, col : col + 1],
                        )
                    # rowmax(ea) == exp(scale*rowmax(logits)); maxp = rowmax*rz
                    col = h * 8 + 2 * itp
                    nc.vector.tensor_reduce(
                        mlbuf[:, col : col + 2],
                        ea[:].rearrange("p (s n) -> p s n", s=2),
                        axis=AX.X,
                        op=ALU.max,
                    )
                    wps2 = psa.tile([128, 512], f32, tag="wps2", name="wps2", bufs=2)
                    nc.tensor.matmul(wps2[:], lhsT=wut[:, 0:128], rhs=ea[:, 0:512], start=True, stop=True)

          # ---- AR#2b: global max prob; ln(r)/c rows ----------------------
          with (
            tc.tile_pool(name="phR", bufs=1) as pr,
            tc.tile_pool(name="psT", bufs=1, space="PSUM") as pst,
          ):
            from concourse.masks import make_identity

            rz = pr.tile([128, 96], f32, tag="rz", name="rz")
            nc.vector.reciprocal(rz[:], zbuf[:])
            mp = pr.tile([128, 96], f32, tag="mp", name="mp")
            nc.vector.tensor_tensor(mp[:], mlbuf[:], rz[:], ALU.mult)
            pk2 = pr.tile([128, 1], f32, tag="pk2", name="pk2")
            nc.vector.tensor_reduce(pk2[:], mp[:], axis=AX.X, op=ALU.max)
            pk2r = pr.tile([128, 1], f32, tag="pk2r", name="pk2r")
            nc.gpsimd.partition_all_reduce(pk2r[:], pk2[:], 128, RED.max)
            nc.gpsimd.dma_start(ar2b_in[:], pk2r[0:1, :])
            nc.gpsimd.collective_compute(
                "AllReduce", ALU.max, replica_groups=RG, ins=[ar2b_in.opt()], outs=[ar2b_out.opt()]
            )
            gmp = pr.tile([1, 1], f32, tag="gmp", name="gmp")
            nc.gpsimd.dma_start(gmp[:], ar2b_out[:])
            nc.gpsimd.partition_broadcast(sc[:, 7:8], gmp[:])

            # v quantize fills the AR#2b latency window (needs only AR#2a):
            # scale+magic on gpsimd, strided round writes directly into vz
            for nt in range(8):
                y = pr.tile([128, C], f32, tag="yv", name="yv", bufs=2)
                nc.vector.tensor_scalar(y[:], v_f[nt][:], sc[:, 15:16], MAGIC, ALU.mult, ALU.add)
                for hp in range(6):
                    h0, h1 = 2 * hp, 2 * hp + 1
                    nc.gpsimd.tensor_scalar(
                        vz[hp][:, nt * 256 : nt * 256 + 64],
                        y[:, h0 * 64 : (h0 + 1) * 64],
                        MAGIC, None, ALU.subtract,
                    )
                    nc.gpsimd.tensor_scalar(
                        vz[hp][:, nt * 256 + 192 : nt * 256 + 256],
                        y[:, h1 * 64 : (h1 + 1) * 64],
                        MAGIC, None, ALU.subtract,
                    )

            # keep-warm during AR#2b
            _warm_burst(nc, pst, pr, wut, pk2r, 30, 2, tag="psT2", shape=[128, 512])

            # r rows (need AR#2b): r = 127*rz/maxp
            nc.vector.reciprocal(sc[:, 8:9], sc[:, 7:8])
            nc.vector.tensor_scalar(sc[:, 8:9], sc[:, 8:9], 127.0, None, ALU.mult)
            rbuf = pr.tile([128, 96], f32, tag="rbuf", name="rbuf")
            nc.vector.tensor_scalar(rbuf[:], rz[:], sc[:, 8:9], None, ALU.mult)
            lnr = pr.tile([128, 96], f32, tag="lnr", name="lnr")
            nc.scalar.activation(lnr[:], rbuf[:], ACT.Ln)
            lnrc = pr.tile([128, 128], f32, tag="lnrc", name="lnrc")
            nc.vector.memset(lnrc[:], 0.0)
            nc.vector.tensor_scalar(lnrc[:, 0:96], lnr[:], sc[:, 9:10], None, ALU.mult)

            idn = pr.tile([128, 128], f32, tag="idn", name="idn")
            make_identity(nc, idn[:])
            psT = pst.tile([128, 128], f32, tag="psT", name="psT")
            nc.tensor.transpose(psT[:], lnrc[:], idn[:])
            lnrcT = pr.tile([128, 128], f32, tag="lnrcT", name="lnrcT")
            nc.scalar.activation(lnrcT[:], psT[:], ACT.Copy)
            lnrc_rows = pr.tile([H, N], f32, tag="lnrc_rows", name="lnrc_rows")
            hi_rows = pr.tile([H, N], bf16, tag="hi_rows", name="hi_rows")
            lo_rows = pr.tile([H, N], bf16, tag="lo_rows", name="lo_rows")
            for h in range(H):
                nc.sync.dma_start(lnrc_rows[h : h + 1, :], lnrcT[h * 8 : (h + 1) * 8, :])
            nc.vector.tensor_copy(hi_rows[:], lnrc_rows[:])
            nc.vector.scalar_tensor_tensor(
                lo_rows[:], lnrc_rows[:], 1.0, hi_rows[:], ALU.mult, ALU.subtract
            )
            for h in range(H):
                nc.sync.dma_start(aug_q[h][64:65, :], hi_rows[h : h + 1, :])
                nc.sync.dma_start(aug_q[h][65:66, :], lo_rows[h : h + 1, :])

         # ---- phase B: quantized probs + PV (persistent padded vz) -------
         with tc.tile_pool(name="oint_pool", bufs=1) as op_:
           o_int = [op_.tile([128, N], f32, tag=f"oint{t}", name=f"oint{t}") for t in range(6)]
           wp_bf = [op_.tile([128, C], bf16, tag=f"wp{t}", name=f"wp{t}") for t in range(6)]
           for t in range(6):
               nc.sync.dma_start(wp_bf[t][:], wq_proj_ext[t * 128 : (t + 1) * 128, :])
           with (
             tc.tile_pool(name="phB", bufs=4) as pb,
             tc.tile_pool(name="psB", bufs=3, space="PSUM") as psb,
             tc.tile_pool(name="psO", bufs=1, space="PSUM") as pso_pool,
           ):
             for hp in range(6):
                 h0, h1 = 2 * hp, 2 * hp + 1
                 pso = pso_pool.tile([128, N], f32, tag="pso", name="pso")
                 for jt in range(8):
                     pqs = []
                     for hh, h in enumerate((h0, h1)):
                         pslT = psb.tile([128, N], f32, tag="pslT", name="pslT")
                         for ic in range(2):
                             mi = nc.tensor.matmul(
                                 pslT[:, ic * 512 : (ic + 1) * 512],
                                 lhsT=aug_k[h][0:66, jt * 128 : (jt + 1) * 128],
                                 rhs=aug_q[h][0:66, ic * 512 : (ic + 1) * 512],
                                 start=True,
                                 stop=True,
                             )
                             if ic == 1:
                                 mi.ins.ldweights = False
                         ep = pb.tile([128, N], f32, tag="ep", name="ep")
                         nc.scalar.activation(ep[:], pslT[:], ACT.Exp, scale=sc[:, 3:4])
                         pq = pb.tile([128, N], bf16, tag="pq", name="pq")
                         if hh == 0:
                             nc.vector.tensor_scalar(pq[:], ep[:], MAGIC, MAGIC, ALU.add, ALU.subtract)
                         else:
                             nc.gpsimd.tensor_scalar(pq[:], ep[:], MAGIC, MAGIC, ALU.add, ALU.subtract)
                         pqs.append(pq)
                     for hh, pq in enumerate(pqs):
                         for ic in range(2):
                             mi = nc.tensor.matmul(
                                 pso[:, ic * 512 : (ic + 1) * 512],
                                 lhsT=vz[hp][:, jt * 256 + hh * 128 : jt * 256 + (hh + 1) * 128],
                                 rhs=pq[:, ic * 512 : (ic + 1) * 512],
                                 start=(jt == 0 and hh == 0),
                                 stop=(jt == 7 and hh == 1),
                                 skip_group_check=True,
                             )
                             if ic == 1:
                                 mi.ins.ldweights = False
                 nc.scalar.activation(o_int[hp][:], pso[:], ACT.Copy)
                 nc.vector.tensor_reduce(
                     o_abs[:, hp : hp + 1], pso[:], axis=AX.X, op=ALU.max, apply_absolute_value=True
                 )

           # ---- AR#3 + quantize o + proj ----------------------------------
           with (
             tc.tile_pool(name="phC", bufs=3) as pc,
             tc.tile_pool(name="oq_pool", bufs=1) as oqp,
             tc.tile_pool(name="psF", bufs=4, space="PSUM") as psf_pool,
           ):
             bprow = pc.tile([1, C], f32, tag="bprow", name="bprow")
             nc.sync.dma_start(bprow[:], bp_ext[:])
             nc.gpsimd.partition_broadcast(bp_bc[:], bprow[:])
             oam = pc.tile([128, 1], f32, tag="oam", name="oam")
             nc.vector.tensor_reduce(oam[:], o_abs[:], axis=AX.X, op=ALU.max)
             oamr = pc.tile([128, 1], f32, tag="oamr", name="oamr")
             nc.gpsimd.partition_all_reduce(oamr[:], oam[:], 128, RED.max)
             nc.gpsimd.dma_start(ar3_in[:], oamr[0:1, :])
             nc.gpsimd.collective_compute(
                 "AllReduce", ALU.max, replica_groups=RG, ins=[ar3_in.opt()], outs=[ar3_out.opt()]
             )
             go = pc.tile([1, 1], f32, tag="go", name="go")
             nc.gpsimd.dma_start(go[:], ar3_out[:])
             nc.gpsimd.partition_broadcast(sc[:, 10:11], go[:])

             # keep-warm during AR#3
             _warm_burst(nc, psf_pool, pc, wut, oamr, 30, 3, tag="psf", shape=[128, C])

             nc.vector.reciprocal(sc[:, 11:12], sc[:, 10:11])
             nc.vector.tensor_scalar(sc[:, 11:12], sc[:, 11:12], 127.0, None, ALU.mult)
             nc.vector.tensor_tensor(sc[:, 12:13], sc[:, 7:8], sc[:, 14:15], ALU.mult)
             nc.vector.tensor_tensor(sc[:, 12:13], sc[:, 12:13], sc[:, 10:11], ALU.mult)
             nc.vector.tensor_scalar(sc[:, 12:13], sc[:, 12:13], SFIN_CONST[0], None, ALU.mult)

             oq = [oqp.tile([128, N], bf16, tag=f"oq{t}", name=f"oq{t}") for t in range(6)]
             for t in range(6):
                 y = pc.tile([128, N], f32, tag="yo", name="yo")
                 nc.scalar.activation(y[:], o_int[t][:], ACT.Identity, bias=magic_col[:], scale=sc[:, 11:12])
                 nc.gpsimd.tensor_scalar(oq[t][:], y[:], MAGIC, None, ALU.subtract)

             for g in range(2):
                 psfs = [psf_pool.tile([128, C], f32, tag="psf", name="psf") for _ in range(4)]
                 for kt in range(6):
                     for nn in range(4):
                         nt = g * 4 + nn
                         for ick2, (ck, cw) in enumerate(((0, 512), (512, 256))):
                             mi = nc.tensor.matmul(
                                 psfs[nn][:, ck : ck + cw],
                                 lhsT=oq[kt][:, nt * 128 : (nt + 1) * 128],
                                 rhs=wp_bf[kt][:, ck : ck + cw],
                                 start=(kt == 0),
                                 stop=(kt == 5),
                                 skip_group_check=True,
                             )
                             if ick2 == 1:
                                 mi.ins.ldweights = False
                 for nn in range(4):
                     nt = g * 4 + nn
                     ot = pc.tile([128, C], f32, tag="ot", name="ot")
                     nc.vector.scalar_tensor_tensor(
                         ot[:], psfs[nn][:], sc[:, 12:13], bp_bc[:], ALU.mult, ALU.add
                     )
                     eng = nc.sync if nn % 2 == 0 else nc.scalar
                     eng.dma_start(out_ext[nt * 128 : (nt + 1) * 128, :], ot[:])


def _host_prep(x, w_qkv, b_qkv, w_proj, b_proj):
    x = np.asarray(x, dtype=np.float32)
    w_qkv = np.asarray(w_qkv, dtype=np.float32)
    b_qkv = np.asarray(b_qkv, dtype=np.float32)
    w_proj = np.asarray(w_proj, dtype=np.float32)
    b_proj = np.asarray(b_proj, dtype=np.float32)

    qmax = np.float32(127.0)
    s_x = np.maximum(np.max(np.abs(x)) / qmax, np.float32(1e-8))
    s_wq = np.maximum(np.max(np.abs(w_qkv)) / qmax, np.float32(1e-8))
    s_wp = np.maximum(np.max(np.abs(w_proj)) / qmax, np.float32(1e-8))
    inv_s_x = float(np.float32(1.0) / s_x)

    wq_qkv = np.round(w_qkv / s_wq).astype(ml_dtypes.bfloat16)
    wq_proj = np.round(w_proj / s_wp).astype(ml_dtypes.bfloat16)
    bqs = (b_qkv / (s_x * s_wq)).astype(np.float32)[None, :]
    bp = b_proj.astype(np.float32)[None, :]

    sxw = float(s_x) * float(s_wq)
    sfin = float(s_wp) * sxw / (127.0**3)
    catt = 0.125 * sxw * sxw / (127.0 * 127.0)
    in_maps = [
        {
            "xT": np.ascontiguousarray(x[b].T),
            "wq_qkv": wq_qkv,
            "wq_proj": wq_proj,
            "bqs": bqs,
            "bp": bp,
        }
        for b in range(B)
    ]
    return inv_s_x, sfin, catt, in_maps


_CACHE = {}


def kernel(x, w_qkv, b_qkv, w_proj, b_proj):
    inv_s_x, sfin, catt, in_maps = _host_prep(x, w_qkv, b_qkv, w_proj, b_proj)
    key = (inv_s_x, sfin, catt)
    if key not in _CACHE:
        SFIN_CONST[0] = sfin
        CATT_CONST[0] = catt
        _CACHE[key] = build_graph(inv_s_x)
    nc = _CACHE[key]
    res = run_bass_kernel_spmd(nc, in_maps, CORES)
    return np.stack([res.results[b]["out"] for b in range(B)], axis=0)


def build_and_inmaps(x, w_qkv, b_qkv, w_proj, b_proj):
    inv_s_x, sfin, catt, in_maps = _host_prep(x, w_qkv, b_qkv, w_proj, b_proj)
    SFIN_CONST[0] = sfin
    CATT_CONST[0] = catt
    nc = build_graph(inv_s_x)
    return nc, in_maps


# revision 17
# speedup vs baseline: 2.7829x; 2.7829x over previous
"""Fake-quantized multi-head attention block on 8 TRN2 NeuronCores.

Data-parallel over batch (1 element per core); integer-domain quantized
matmuls in bf16; global fake-quant scales via tiny AllReduce(max)
collectives. Engine-balanced v2: rowmax tree on gpsimd, split AR#2,
persistent padded-V tiles, [0:66]-sliced aug operands.
"""

import sys

sys.path.insert(0, "/opt/trn_rl_repo")

import numpy as np
import ml_dtypes

import concourse.mybir as mybir
import concourse.tile as tile
import concourse.bass_isa as bass_isa
from concourse import bacc
from concourse.bass_utils import run_bass_kernel_spmd

f32 = mybir.dt.float32
bf16 = mybir.dt.bfloat16
ALU = mybir.AluOpType
ACT = mybir.ActivationFunctionType
AX = mybir.AxisListType
RED = bass_isa.ReduceOp

B, N, C = 8, 1024, 768
H, HD = 12, 64
NCORES = 8
MAGIC = float(np.float32(3 * 2**22))
CORES = list(range(NCORES))
RG = [CORES]

SFIN_CONST = [1.0]
CATT_CONST = [1.0]


def _warm_burst(nc, pool, sbpool, wut, anchor, n, tagix, tag, shape, bufs=None):
    """Keep-warm dummy matmuls anchored to a late-ready tile so they run
    exactly during a collective's latency window (PE otherwise idle)."""
    anch = sbpool.tile([128, 512], bf16, tag=f"anc{tagix}", name=f"anc{tagix}")
    nc.vector.memset(anch[:], 1.0)
    w = min(512, anchor.shape[-1])
    nc.vector.tensor_copy(anch[:, 0:w], anchor[:, 0:w])
    wps = pool.tile(shape, f32, tag=tag, name=f"wub{tagix}", bufs=bufs)
    wps2 = pool.tile(shape, f32, tag=tag, name=f"wub{tagix}b", bufs=bufs)
    for i in range(n):
        nc.tensor.matmul(
            (wps if i % 2 == 0 else wps2)[:, 0:512],
            lhsT=wut[:, 0:128], rhs=anch[:], start=True, stop=True,
        )


def build_graph(inv_s_x: float):
    nc = bacc.Bacc("TRN2", target_bir_lowering=False, debug=False, num_devices=NCORES)

    xT_ext = nc.dram_tensor("xT", [C, N], f32, kind="ExternalInput")
    wq_qkv_ext = nc.dram_tensor("wq_qkv", [C, 3 * C], bf16, kind="ExternalInput")
    wq_proj_ext = nc.dram_tensor("wq_proj", [C, C], bf16, kind="ExternalInput")
    bqs_ext = nc.dram_tensor("bqs", [1, 3 * C], f32, kind="ExternalInput")
    bp_ext = nc.dram_tensor("bp", [1, C], f32, kind="ExternalInput")
    out_ext = nc.dram_tensor("out", [N, C], f32, kind="ExternalOutput")

    with tile.TileContext(nc) as tc:
        run_body(nc, tc, inv_s_x, xT_ext, wq_qkv_ext, wq_proj_ext, bqs_ext, bp_ext, out_ext)
    nc.finalize()
    return nc


def run_body(nc, tc, inv_s_x, xT_ext, wq_qkv_ext, wq_proj_ext, bqs_ext, bp_ext, out_ext):
    with (
        tc.tile_pool(name="persist", bufs=1) as pp,
        tc.tile_pool(name="dram", bufs=1, space="DRAM") as dram,
    ):
        # aug tensors: rows 0:64 = q/k head slices (ints); aug_k rows 64:66 =
        # ones; aug_q rows 64:66 = ln(r)/c hi/lo (zero during phase A, written
        # after AR#2b). All attention matmuls slice partitions [0:66], so rows
        # 66:128 are never initialized.
        aug_q = [pp.tile([128, N], bf16, tag=f"augq{h}", name=f"augq{h}") for h in range(H)]
        aug_k = [pp.tile([128, N], bf16, tag=f"augk{h}", name=f"augk{h}") for h in range(H)]
        zbuf = pp.tile([128, 96], f32, tag="zbuf", name="zbuf")
        mlbuf = pp.tile([128, 96], f32, tag="mlbuf", name="mlbuf")  # rowmax of ea
        qkv_abs = pp.tile([128, 24], f32, tag="qkv_abs", name="qkv_abs")
        v_abs = pp.tile([128, 16], f32, tag="v_abs", name="v_abs")
        o_abs = pp.tile([128, 6], f32, tag="o_abs", name="o_abs")
        sc = pp.tile([128, 16], f32, tag="sc", name="sc")
        bqs_cols = pp.tile([128, 12], f32, tag="bqs_cols", name="bqs_cols")
        bv_bc = pp.tile([128, C], f32, tag="bv_bc", name="bv_bc")
        magic_col = pp.tile([128, 1], f32, tag="magic_col", name="magic_col")

        ar1_in = dram.tile([1, 2], f32, tag="ar1_in", name="ar1_in")
        ar1_out = dram.tile([1, 2], f32, tag="ar1_out", name="ar1_out")
        ar2a_in = dram.tile([1, 1], f32, tag="ar2a_in", name="ar2a_in")
        ar2a_out = dram.tile([1, 1], f32, tag="ar2a_out", name="ar2a_out")
        ar2b_in = dram.tile([1, 1], f32, tag="ar2b_in", name="ar2b_in")
        ar2b_out = dram.tile([1, 1], f32, tag="ar2b_out", name="ar2b_out")
        ar3_in = dram.tile([1, 1], f32, tag="ar3_in", name="ar3_in")
        ar3_out = dram.tile([1, 1], f32, tag="ar3_out", name="ar3_out")

        # PE warm-up burst first (flip HAM to 8/8)
        wut = pp.tile([128, 512], bf16, tag="wut", name="wut")
        nc.vector.memset(wut[:], 1.0)
        with tc.tile_pool(name="pswu", bufs=1, space="PSUM") as pswu:
            wps = pswu.tile([128, 512], f32, tag="wps", name="wps")
            for _ in range(20):
                nc.tensor.matmul(wps[:], lhsT=wut[:, 0:128], rhs=wut[:], start=True, stop=True)
        nc.vector.memset(magic_col[:], MAGIC)
        for h in range(H):
            nc.vector.memset(aug_q[h][64:66, :], 0.0)
            nc.vector.memset(aug_k[h][64:66, :], 1.0)
        with tc.tile_pool(name="brow", bufs=1) as br:
            bvrow = br.tile([1, C], f32, tag="bvrow", name="bvrow")
            nc.sync.dma_start(bvrow[:], bqs_ext[0:1, 1536:2304])
            nc.gpsimd.partition_broadcast(bv_bc[:], bvrow[:])

        # late pool: tensors born mid-kernel (vz at v-quant, bp_bc at proj)
        with tc.tile_pool(name="late", bufs=1) as lp:
         # padded PV weights, persistent: vz[hp][:, jt*256 + 0:64] = v cols of
         # head 2hp (key-block jt), [jt*256+192:256] = head 2hp+1, gaps zero.
         vz = [lp.tile([128, 8 * 256], bf16, tag=f"vz{t}", name=f"vz{t}") for t in range(6)]
         bp_bc = lp.tile([128, C], f32, tag="bp_bc", name="bp_bc")
         # v_f persists until v-quantize (after AR#2a)
         with tc.tile_pool(name="vf_pool", bufs=1) as vfp:
          v_f = [vfp.tile([128, C], f32, tag=f"vf{t}", name=f"vf{t}") for t in range(8)]

          # ---- stage 1+2: x quant, QKV matmuls, AR#1 (q,k), quantize -----
          with (
            tc.tile_pool(name="wload", bufs=1) as wl,
            tc.tile_pool(name="qkvf_pool", bufs=1) as qp,
            tc.tile_pool(name="s12", bufs=2) as s12,
            tc.tile_pool(name="psq", bufs=4, space="PSUM") as psq,
          ):
            wq_bf = [wl.tile([128, 3 * C], bf16, tag=f"wq{t}", name=f"wq{t}") for t in range(6)]
            xq = [wl.tile([128, N], bf16, tag=f"xq{t}", name=f"xq{t}") for t in range(6)]
            qkv_f = [qp.tile([128, N], f32, tag=f"qkvf{t}", name=f"qkvf{t}") for t in range(12)]

            for t in range(6):
                xf = s12.tile([128, N], f32, tag="s12y", name="xf")
                nc.sync.dma_start(xf[:], xT_ext[t * 128 : (t + 1) * 128, :])
                nc.sync.dma_start(wq_bf[t][:], wq_qkv_ext[t * 128 : (t + 1) * 128, :])
                # x quantize: scale+magic on Act (idle in this window), round on DVE
                y = s12.tile([128, N], f32, tag="s12y", name="y")
                nc.scalar.activation(y[:], xf[:], ACT.Identity, bias=magic_col[:], scale=inv_s_x)
                nc.vector.tensor_scalar(xq[t][:], y[:], MAGIC, None, ALU.subtract)
            for mt in range(12):
                nc.sync.dma_start(
                    bqs_cols[:, mt : mt + 1],
                    bqs_ext[0:1, mt * 128 : (mt + 1) * 128].rearrange("a (p b) -> (a p) b", b=1),
                )

            # q/k part: per-mt accumulation over kt
            for mt in range(12):
                pss = [psq.tile([128, 512], f32, tag="psq", name="psq", bufs=6) for _ in range(2)]
                for kt in range(6):
                    for nk in range(2):
                        mi = nc.tensor.matmul(
                            pss[nk][:],
                            lhsT=wq_bf[kt][:, mt * 128 : (mt + 1) * 128],
                            rhs=xq[kt][:, nk * 512 : (nk + 1) * 512],
                            start=(kt == 0),
                            stop=(kt == 5),
                            skip_group_check=True,
                        )
                        if nk == 1:
                            mi.ins.ldweights = False
                for nk in range(2):
                    # bias-add + PSUM->SBUF on Act; absmax post-bias on DVE
                    nc.scalar.activation(
                        qkv_f[mt][:, nk * 512 : (nk + 1) * 512],
                        pss[nk][:],
                        ACT.Identity,
                        bias=bqs_cols[:, mt : mt + 1],
                    )
                    nc.vector.tensor_reduce(
                        qkv_abs[:, mt * 2 + nk : mt * 2 + nk + 1],
                        qkv_f[mt][:, nk * 512 : (nk + 1) * 512],
                        axis=AX.X,
                        op=ALU.max,
                        apply_absolute_value=True,
                    )

            # ---- AR#1: global absmax of q, k only --------------------------
            # (absmax computed pre-bias on PSUM would be wrong; bias shifts
            # values. Reduce from qkv_f instead -- see note below.)
            am2 = s12.tile([128, 2], f32, tag="am2", name="am2")
            nc.vector.tensor_reduce(am2[:, 0:1], qkv_abs[:, 0:12], axis=AX.X, op=ALU.max)
            nc.vector.tensor_reduce(am2[:, 1:2], qkv_abs[:, 12:24], axis=AX.X, op=ALU.max)
            am2r = s12.tile([128, 2], f32, tag="am2r", name="am2r")
            nc.gpsimd.partition_all_reduce(am2r[:], am2[:], 128, RED.max)
            nc.gpsimd.dma_start(ar1_in[:], am2r[0:1, :])
            nc.gpsimd.collective_compute(
                "AllReduce", ALU.max, replica_groups=RG, ins=[ar1_in.opt()], outs=[ar1_out.opt()]
            )
            # keep-warm during AR#1 (runs alongside the v matmuls below)
            _warm_burst(nc, psq, s12, wut, am2r, 20, 1, tag="psq", shape=[128, 512], bufs=6)
            g2 = pp.tile([1, 2], f32, tag="g2", name="g2")
            nc.gpsimd.dma_start(g2[:], ar1_out[:])
            g2b = pp.tile([128, 2], f32, tag="g2b", name="g2b")
            nc.gpsimd.partition_broadcast(g2b[:], g2[:])

            inv2 = pp.tile([128, 2], f32, tag="inv2", name="inv2")
            nc.vector.reciprocal(inv2[:], g2b[:])
            nc.vector.tensor_scalar(inv2[:], inv2[:], 127.0, None, ALU.mult)
            nc.vector.tensor_tensor(sc[:, 3:4], g2b[:, 0:1], g2b[:, 1:2], ALU.mult)
            nc.vector.tensor_scalar(sc[:, 3:4], sc[:, 3:4], CATT_CONST[0], None, ALU.mult)
            nc.vector.reciprocal(sc[:, 9:10], sc[:, 3:4])
            # v matmuls fill the PE idle window during/after AR#1
            for nt in range(8):
                pss = []
                for ick, (ck, cw) in enumerate(((0, 512), (512, 256))):
                    pss.append((psq.tile([128, 512], f32, tag="psq", name="psv", bufs=6), ck, cw))
                for kt in range(6):
                    for ick2, (ps, ck, cw) in enumerate(pss):
                        mi = nc.tensor.matmul(
                            ps[:, 0:cw],
                            lhsT=xq[kt][:, nt * 128 : (nt + 1) * 128],
                            rhs=wq_bf[kt][:, 1536 + ck : 1536 + ck + cw],
                            start=(kt == 0),
                            stop=(kt == 5),
                            skip_group_check=True,
                        )
                        if ick2 == 1:
                            mi.ins.ldweights = False
                for ick, (ps, ck, cw) in enumerate(pss):
                    nc.vector.scalar_tensor_tensor(
                        v_f[nt][:, ck : ck + cw],
                        ps[:, 0:cw],
                        1.0,
                        bv_bc[:, ck : ck + cw],
                        ALU.mult,
                        ALU.add,
                    )
                    nc.vector.tensor_reduce(
                        v_abs[:, nt * 2 + ick : nt * 2 + ick + 1],
                        v_f[nt][:, ck : ck + cw],
                        axis=AX.X,
                        op=ALU.max,
                        apply_absolute_value=True,
                    )

            # ---- AR#2a: global absmax of v, issued EARLY (hides under
            # phase A; result needed only at v-quantize) -------------------
            vam = s12.tile([128, 1], f32, tag="vam", name="vam")
            nc.vector.tensor_reduce(vam[:], v_abs[:], axis=AX.X, op=ALU.max)
            vamr = s12.tile([128, 1], f32, tag="vamr", name="vamr")
            nc.gpsimd.partition_all_reduce(vamr[:], vam[:], 128, RED.max)
            nc.gpsimd.dma_start(ar2a_in[:], vamr[0:1, :])
            nc.gpsimd.collective_compute(
                "AllReduce", ALU.max, replica_groups=RG, ins=[ar2a_in.opt()], outs=[ar2a_out.opt()]
            )
            gmv = pp.tile([1, 1], f32, tag="gmv", name="gmv")
            nc.gpsimd.dma_start(gmv[:], ar2a_out[:])
            nc.gpsimd.partition_broadcast(sc[:, 14:15], gmv[:])
            nc.vector.reciprocal(sc[:, 15:16], sc[:, 14:15])
            nc.vector.tensor_scalar(sc[:, 15:16], sc[:, 15:16], 127.0, None, ALU.mult)

            # ---- quantize q/k; build aug tensors ---------------------------
            # op1 (scale+magic, AP scalar) on DVE; op2 (round, imm) on gpsimd
            for mt in (0, 6, 1, 7, 2, 8, 3, 9, 4, 10, 5, 11):
                inv = inv2[:, 0:1] if mt < 6 else inv2[:, 1:2]
                y = s12.tile([128, N], f32, tag="s12y", name="yq")
                nc.vector.tensor_scalar(y[:], qkv_f[mt][:], inv, MAGIC, ALU.mult, ALU.add)
                qsc = s12.tile([128, N], bf16, tag="qsc", name="qsc")
                nc.vector.tensor_scalar(qsc[:], y[:], MAGIC, None, ALU.subtract)
                dst = aug_q if mt < 6 else aug_k
                tt = mt if mt < 6 else mt - 6
                nc.sync.dma_start(dst[2 * tt][0:64, :], qsc[0:64, :])
                nc.sync.dma_start(dst[2 * tt + 1][0:64, :], qsc[64:128, :])

            # zero-fill vz gaps on gpsimd (idle until phase B; must precede
            # the strided v-round writes at AR#2b)
            for t in range(6):
                nc.gpsimd.memset(vz[t][:], 0.0)

          # ---- phase A: attn logits stats (z via Act accum, rowmax of ea
          # via gpsimd max-tree + small DVE reduce) ------------------------
          with (
            tc.tile_pool(name="phA", bufs=4) as pa,
            tc.tile_pool(name="psA", bufs=3, space="PSUM") as psa,
          ):
            for h in range(H):
                for itp in range(4):
                    # paired it-blocks: one [128, 2N] ea tile, one DVE rowmax
                    ea = pa.tile([128, 2 * N], bf16, tag="ea", name="ea")
                    for s in range(2):
                        it = 2 * itp + s
                        psl = psa.tile([128, N], f32, tag="psl", name="psl")
                        for jc in range(2):
                            mi = nc.tensor.matmul(
                                psl[:, jc * 512 : (jc + 1) * 512],
                                lhsT=aug_q[h][0:66, it * 128 : (it + 1) * 128],
                                rhs=aug_k[h][0:66, jc * 512 : (jc + 1) * 512],
                                start=True,
                                stop=True,
                            )
                            if jc == 1:
                                mi.ins.ldweights = False
                        col = h * 8 + it
                        nc.scalar.activation(
                            ea[:, s * N : (s + 1) * N], psl[:], ACT.Exp, scale=sc[:, 3:4],
                            accum_out=zbuf[:, col : col + 1],
                        )
                    # rowmax(ea) == exp(scale*rowmax(logits)); maxp = rowmax*rz
                    col = h * 8 + 2 * itp
                    nc.vector.tensor_reduce(
                        mlbuf[:, col : col + 2],
                        ea[:].rearrange("p (s n) -> p s n", s=2),
                        axis=AX.X,
                        op=ALU.max,
                    )
                    wps2 = psa.tile([128, 512], f32, tag="wps2", name="wps2", bufs=2)
                    nc.tensor.matmul(wps2[:], lhsT=wut[:, 0:128], rhs=ea[:, 0:512], start=True, stop=True)

          # ---- AR#2b: global max prob; ln(r)/c rows ----------------------
          with (
            tc.tile_pool(name="phR", bufs=1) as pr,
            tc.tile_pool(name="psT", bufs=1, space="PSUM") as pst,
          ):
            from concourse.masks import make_identity

            rz = pr.tile([128, 96], f32, tag="rz", name="rz")
            nc.vector.reciprocal(rz[:], zbuf[:])
            mp = pr.tile([128, 96], f32, tag="mp", name="mp")
            nc.vector.tensor_tensor(mp[:], mlbuf[:], rz[:], ALU.mult)
            pk2 = pr.tile([128, 1], f32, tag="pk2", name="pk2")
            nc.vector.tensor_reduce(pk2[:], mp[:], axis=AX.X, op=ALU.max)
            pk2r = pr.tile([128, 1], f32, tag="pk2r", name="pk2r")
            nc.gpsimd.partition_all_reduce(pk2r[:], pk2[:], 128, RED.max)
            nc.gpsimd.dma_start(ar2b_in[:], pk2r[0:1, :])
            nc.gpsimd.collective_compute(
                "AllReduce", ALU.max, replica_groups=RG, ins=[ar2b_in.opt()], outs=[ar2b_out.opt()]
            )
            gmp = pr.tile([1, 1], f32, tag="gmp", name="gmp")
            nc.gpsimd.dma_start(gmp[:], ar2b_out[:])
            nc.gpsimd.partition_broadcast(sc[:, 7:8], gmp[:])

            # v quantize fills the AR#2b latency window (needs only AR#2a):
            # scale+magic on gpsimd, strided round writes directly into vz
            for nt in range(8):
                y = pr.tile([128, C], f32, tag="yv", name="yv", bufs=2)
                nc.vector.tensor_scalar(y[:], v_f[nt][:], sc[:, 15:16], MAGIC, ALU.mult, ALU.add)
                for hp in range(6):
                    h0, h1 = 2 * hp, 2 * hp + 1
                    nc.vector.tensor_scalar(
                        vz[hp][:, nt * 256 : nt * 256 + 64],
                        y[:, h0 * 64 : (h0 + 1) * 64],
                        MAGIC, None, ALU.subtract,
                    )
                    nc.vector.tensor_scalar(
                        vz[hp][:, nt * 256 + 192 : nt * 256 + 256],
                        y[:, h1 * 64 : (h1 + 1) * 64],
                        MAGIC, None, ALU.subtract,
                    )

            # keep-warm during AR#2b
            _warm_burst(nc, pst, pr, wut, pk2r, 30, 2, tag="psT2", shape=[128, 512])

            # r rows (need AR#2b): r = 127*rz/maxp
            nc.vector.reciprocal(sc[:, 8:9], sc[:, 7:8])
            nc.vector.tensor_scalar(sc[:, 8:9], sc[:, 8:9], 127.0, None, ALU.mult)
            rbuf = pr.tile([128, 96], f32, tag="rbuf", name="rbuf")
            nc.vector.tensor_scalar(rbuf[:], rz[:], sc[:, 8:9], None, ALU.mult)
            lnr = pr.tile([128, 96], f32, tag="lnr", name="lnr")
            nc.scalar.activation(lnr[:], rbuf[:], ACT.Ln)
            lnrc = pr.tile([128, 128], f32, tag="lnrc", name="lnrc")
            nc.vector.memset(lnrc[:], 0.0)
            nc.vector.tensor_scalar(lnrc[:, 0:96], lnr[:], sc[:, 9:10], None, ALU.mult)

            idn = pr.tile([128, 128], f32, tag="idn", name="idn")
            make_identity(nc, idn[:])
            psT = pst.tile([128, 128], f32, tag="psT", name="psT")
            nc.tensor.transpose(psT[:], lnrc[:], idn[:])
            lnrcT = pr.tile([128, 128], f32, tag="lnrcT", name="lnrcT")
            nc.scalar.activation(lnrcT[:], psT[:], ACT.Copy)
            lnrc_rows = pr.tile([H, N], f32, tag="lnrc_rows", name="lnrc_rows")
            hi_rows = pr.tile([H, N], bf16, tag="hi_rows", name="hi_rows")
            lo_rows = pr.tile([H, N], bf16, tag="lo_rows", name="lo_rows")
            for h in range(H):
                nc.sync.dma_start(lnrc_rows[h : h + 1, :], lnrcT[h * 8 : (h + 1) * 8, :])
            nc.vector.tensor_copy(hi_rows[:], lnrc_rows[:])
            nc.vector.scalar_tensor_tensor(
                lo_rows[:], lnrc_rows[:], 1.0, hi_rows[:], ALU.mult, ALU.subtract
            )
            for h in range(H):
                nc.sync.dma_start(aug_q[h][64:65, :], hi_rows[h : h + 1, :])
                nc.sync.dma_start(aug_q[h][65:66, :], lo_rows[h : h + 1, :])

         # ---- phase B: quantized probs + PV (persistent padded vz) -------
         with tc.tile_pool(name="oint_pool", bufs=1) as op_:
           o_int = [op_.tile([128, N], f32, tag=f"oint{t}", name=f"oint{t}") for t in range(6)]
           wp_bf = [op_.tile([128, C], bf16, tag=f"wp{t}", name=f"wp{t}") for t in range(6)]
           for t in range(6):
               nc.sync.dma_start(wp_bf[t][:], wq_proj_ext[t * 128 : (t + 1) * 128, :])
           with (
             tc.tile_pool(name="phB", bufs=4) as pb,
             tc.tile_pool(name="psB", bufs=3, space="PSUM") as psb,
             tc.tile_pool(name="psO", bufs=1, space="PSUM") as pso_pool,
           ):
             for hp in range(6):
                 h0, h1 = 2 * hp, 2 * hp + 1
                 pso = pso_pool.tile([128, N], f32, tag="pso", name="pso")
                 for jt in range(8):
                     pqs = []
                     for hh, h in enumerate((h0, h1)):
                         pslT = psb.tile([128, N], f32, tag="pslT", name="pslT")
                         for ic in range(2):
                             mi = nc.tensor.matmul(
                                 pslT[:, ic * 512 : (ic + 1) * 512],
                                 lhsT=aug_k[h][0:66, jt * 128 : (jt + 1) * 128],
                                 rhs=aug_q[h][0:66, ic * 512 : (ic + 1) * 512],
                                 start=True,
                                 stop=True,
                             )
                             if ic == 1:
                                 mi.ins.ldweights = False
                         ep = pb.tile([128, N], f32, tag="ep", name="ep")
                         nc.scalar.activation(ep[:], pslT[:], ACT.Exp, scale=sc[:, 3:4])
                         pq = pb.tile([128, N], bf16, tag="pq", name="pq")
                         nc.vector.tensor_scalar(pq[:], ep[:], MAGIC, MAGIC, ALU.add, ALU.subtract)
                         pqs.append(pq)
                     for hh, pq in enumerate(pqs):
                         for ic in range(2):
                             mi = nc.tensor.matmul(
                                 pso[:, ic * 512 : (ic + 1) * 512],
                                 lhsT=vz[hp][:, jt * 256 + hh * 128 : jt * 256 + (hh + 1) * 128],
                                 rhs=pq[:, ic * 512 : (ic + 1) * 512],
                                 start=(jt == 0 and hh == 0),
                                 stop=(jt == 7 and hh == 1),
                                 skip_group_check=True,
                             )
                             if ic == 1:
                                 mi.ins.ldweights = False
                 nc.vector.tensor_copy(o_int[hp][:], pso[:])
                 nc.vector.tensor_reduce(
                     o_abs[:, hp : hp + 1], pso[:], axis=AX.X, op=ALU.max, apply_absolute_value=True
                 )

           # ---- AR#3 + quantize o + proj ----------------------------------
           with (
             tc.tile_pool(name="phC", bufs=3) as pc,
             tc.tile_pool(name="oq_pool", bufs=1) as oqp,
             tc.tile_pool(name="psF", bufs=4, space="PSUM") as psf_pool,
           ):
             bprow = pc.tile([1, C], f32, tag="bprow", name="bprow")
             nc.sync.dma_start(bprow[:], bp_ext[:])
             nc.gpsimd.partition_broadcast(bp_bc[:], bprow[:])
             oam = pc.tile([128, 1], f32, tag="oam", name="oam")
             nc.vector.tensor_reduce(oam[:], o_abs[:], axis=AX.X, op=ALU.max)
             oamr = pc.tile([128, 1], f32, tag="oamr", name="oamr")
             nc.gpsimd.partition_all_reduce(oamr[:], oam[:], 128, RED.max)
             nc.gpsimd.dma_start(ar3_in[:], oamr[0:1, :])
             nc.gpsimd.collective_compute(
                 "AllReduce", ALU.max, replica_groups=RG, ins=[ar3_in.opt()], outs=[ar3_out.opt()]
             )
             go = pc.tile([1, 1], f32, tag="go", name="go")
             nc.gpsimd.dma_start(go[:], ar3_out[:])
             nc.gpsimd.partition_broadcast(sc[:, 10:11], go[:])

             # keep-warm during AR#3
             _warm_burst(nc, psf_pool, pc, wut, oamr, 30, 3, tag="psf", shape=[128, C])

             nc.vector.reciprocal(sc[:, 11:12], sc[:, 10:11])
             nc.vector.tensor_scalar(sc[:, 11:12], sc[:, 11:12], 127.0, None, ALU.mult)
             nc.vector.tensor_tensor(sc[:, 12:13], sc[:, 7:8], sc[:, 14:15], ALU.mult)
             nc.vector.tensor_tensor(sc[:, 12:13], sc[:, 12:13], sc[:, 10:11], ALU.mult)
             nc.vector.tensor_scalar(sc[:, 12:13], sc[:, 12:13], SFIN_CONST[0], None, ALU.mult)

             oq = [oqp.tile([128, N], bf16, tag=f"oq{t}", name=f"oq{t}") for t in range(6)]
             for t in range(6):
                 y = pc.tile([128, N], f32, tag="yo", name="yo")
                 nc.scalar.activation(y[:], o_int[t][:], ACT.Identity, bias=magic_col[:], scale=sc[:, 11:12])
                 nc.vector.tensor_scalar(oq[t][:], y[:], MAGIC, None, ALU.subtract)

             for g in range(2):
                 psfs = [psf_pool.tile([128, C], f32, tag="psf", name="psf") for _ in range(4)]
                 for kt in range(6):
                     for nn in range(4):
                         nt = g * 4 + nn
                         for ick2, (ck, cw) in enumerate(((0, 512), (512, 256))):
                             mi = nc.tensor.matmul(
                                 psfs[nn][:, ck : ck + cw],
                                 lhsT=oq[kt][:, nt * 128 : (nt + 1) * 128],
                                 rhs=wp_bf[kt][:, ck : ck + cw],
                                 start=(kt == 0),
                                 stop=(kt == 5),
                                 skip_group_check=True,
                             )
                             if ick2 == 1:
                                 mi.ins.ldweights = False
                 for nn in range(4):
                     nt = g * 4 + nn
                     ot = pc.tile([128, C], f32, tag="ot", name="ot")
                     nc.vector.scalar_tensor_tensor(
                         ot[:], psfs[nn][:], sc[:, 12:13], bp_bc[:], ALU.mult, ALU.add
                     )
                     eng = nc.sync if nn % 2 == 0 else nc.scalar
                     eng.dma_start(out_ext[nt * 128 : (nt + 1) * 128, :], ot[:])


def _host_prep(x, w_qkv, b_qkv, w_proj, b_proj):
    x = np.asarray(x, dtype=np.float32)
    w_qkv = np.asarray(w_qkv, dtype=np.float32)
    b_qkv = np.asarray(b_qkv, dtype=np.float32)
    w_proj = np.asarray(w_proj, dtype=np.float32)
    b_proj = np.asarray(b_proj, dtype=np.float32)

    qmax = np.float32(127.0)
    s_x = np.maximum(np.max(np.abs(x)) / qmax, np.float32(1e-8))
    s_wq = np.maximum(np.max(np.abs(w_qkv)) / qmax, np.float32(1e-8))
    s_wp = np.maximum(np.max(np.abs(w_proj)) / qmax, np.float32(1e-8))
    inv_s_x = float(np.float32(1.0) / s_x)

    wq_qkv = np.round(w_qkv / s_wq).astype(ml_dtypes.bfloat16)
    wq_proj = np.round(w_proj / s_wp).astype(ml_dtypes.bfloat16)
    bqs = (b_qkv / (s_x * s_wq)).astype(np.float32)[None, :]
    bp = b_proj.astype(np.float32)[None, :]

    sxw = float(s_x) * float(s_wq)
    sfin = float(s_wp) * sxw / (127.0**3)
    catt = 0.125 * sxw * sxw / (127.0 * 127.0)
    in_maps = [
        {
            "xT": np.ascontiguousarray(x[b].T),
            "wq_qkv": wq_qkv,
            "wq_proj": wq_proj,
            "bqs": bqs,
            "bp": bp,
        }
        for b in range(B)
    ]
    return inv_s_x, sfin, catt, in_maps


_CACHE = {}


def kernel(x, w_qkv, b_qkv, w_proj, b_proj):
    inv_s_x, sfin, catt, in_maps = _host_prep(x, w_qkv, b_qkv, w_proj, b_proj)
    key = (inv_s_x, sfin, catt)
    if key not in _CACHE:
        SFIN_CONST[0] = sfin
        CATT_CONST[0] = catt
        _CACHE[key] = build_graph(inv_s_x)
    nc = _CACHE[key]
    res = run_bass_kernel_spmd(nc, in_maps, CORES)
    return np.stack([res.results[b]["out"] for b in range(B)], axis=0)


def build_and_inmaps(x, w_qkv, b_qkv, w_proj, b_proj):
    inv_s_x, sfin, catt, in_maps = _host_prep(x, w_qkv, b_qkv, w_proj, b_proj)
    SFIN_CONST[0] = sfin
    CATT_CONST[0] = catt
    nc = build_graph(inv_s_x)
    return nc, in_maps


# revision 18
# speedup vs baseline: 2.8016x; 1.0067x over previous
"""Fake-quantized multi-head attention block on 8 TRN2 NeuronCores.

Data-parallel over batch (1 element per core); integer-domain quantized
matmuls in bf16; global fake-quant scales via tiny AllReduce(max)
collectives. Engine-balanced v2: rowmax tree on gpsimd, split AR#2,
persistent padded-V tiles, [0:66]-sliced aug operands.
"""

import sys

sys.path.insert(0, "/opt/trn_rl_repo")

import numpy as np
import ml_dtypes

import concourse.mybir as mybir
import concourse.tile as tile
import concourse.bass_isa as bass_isa
from concourse import bacc
from concourse.bass_utils import run_bass_kernel_spmd

f32 = mybir.dt.float32
bf16 = mybir.dt.bfloat16
ALU = mybir.AluOpType
ACT = mybir.ActivationFunctionType
AX = mybir.AxisListType
RED = bass_isa.ReduceOp

B, N, C = 8, 1024, 768
H, HD = 12, 64
NCORES = 8
MAGIC = float(np.float32(3 * 2**22))
CORES = list(range(NCORES))
RG = [CORES]

SFIN_CONST = [1.0]
CATT_CONST = [1.0]


def _warm_burst(nc, pool, sbpool, wut, anchor, n, tagix, tag, shape, bufs=None):
    """Keep-warm dummy matmuls anchored to a late-ready tile so they run
    exactly during a collective's latency window (PE otherwise idle)."""
    anch = sbpool.tile([128, 512], bf16, tag=f"anc{tagix}", name=f"anc{tagix}")
    nc.vector.memset(anch[:], 1.0)
    w = min(512, anchor.shape[-1])
    nc.vector.tensor_copy(anch[:, 0:w], anchor[:, 0:w])
    wps = pool.tile(shape, f32, tag=tag, name=f"wub{tagix}", bufs=bufs)
    wps2 = pool.tile(shape, f32, tag=tag, name=f"wub{tagix}b", bufs=bufs)
    for i in range(n):
        nc.tensor.matmul(
            (wps if i % 2 == 0 else wps2)[:, 0:512],
            lhsT=wut[:, 0:128], rhs=anch[:], start=True, stop=True,
        )


def build_graph(inv_s_x: float):
    nc = bacc.Bacc("TRN2", target_bir_lowering=False, debug=False, num_devices=NCORES)

    xT_ext = nc.dram_tensor("xT", [C, N], f32, kind="ExternalInput")
    wq_qkv_ext = nc.dram_tensor("wq_qkv", [C, 3 * C], bf16, kind="ExternalInput")
    wq_proj_ext = nc.dram_tensor("wq_proj", [C, C], bf16, kind="ExternalInput")
    bqs_ext = nc.dram_tensor("bqs", [1, 3 * C], f32, kind="ExternalInput")
    bp_ext = nc.dram_tensor("bp", [1, C], f32, kind="ExternalInput")
    out_ext = nc.dram_tensor("out", [N, C], f32, kind="ExternalOutput")

    with tile.TileContext(nc) as tc:
        run_body(nc, tc, inv_s_x, xT_ext, wq_qkv_ext, wq_proj_ext, bqs_ext, bp_ext, out_ext)
    nc.finalize()
    return nc


def run_body(nc, tc, inv_s_x, xT_ext, wq_qkv_ext, wq_proj_ext, bqs_ext, bp_ext, out_ext):
    with (
        tc.tile_pool(name="persist", bufs=1) as pp,
        tc.tile_pool(name="dram", bufs=1, space="DRAM") as dram,
    ):
        # aug tensors: rows 0:64 = q/k head slices (ints); aug_k rows 64:66 =
        # ones; aug_q rows 64:66 = ln(r)/c hi/lo (zero during phase A, written
        # after AR#2b). All attention matmuls slice partitions [0:66], so rows
        # 66:128 are never initialized.
        aug_q = [pp.tile([128, N], bf16, tag=f"augq{h}", name=f"augq{h}") for h in range(H)]
        aug_k = [pp.tile([128, N], bf16, tag=f"augk{h}", name=f"augk{h}") for h in range(H)]
        zbuf = pp.tile([128, 96], f32, tag="zbuf", name="zbuf")
        mlbuf = pp.tile([128, 96], f32, tag="mlbuf", name="mlbuf")  # rowmax of ea
        qkv_abs = pp.tile([128, 24], f32, tag="qkv_abs", name="qkv_abs")
        v_abs = pp.tile([128, 16], f32, tag="v_abs", name="v_abs")
        o_abs = pp.tile([128, 6], f32, tag="o_abs", name="o_abs")
        sc = pp.tile([128, 16], f32, tag="sc", name="sc")
        bqs_cols = pp.tile([128, 12], f32, tag="bqs_cols", name="bqs_cols")
        bv_bc = pp.tile([128, C], f32, tag="bv_bc", name="bv_bc")
        magic_col = pp.tile([128, 1], f32, tag="magic_col", name="magic_col")

        ar1_in = dram.tile([1, 2], f32, tag="ar1_in", name="ar1_in")
        ar1_out = dram.tile([1, 2], f32, tag="ar1_out", name="ar1_out")
        ar2a_in = dram.tile([1, 1], f32, tag="ar2a_in", name="ar2a_in")
        ar2a_out = dram.tile([1, 1], f32, tag="ar2a_out", name="ar2a_out")
        ar2b_in = dram.tile([1, 1], f32, tag="ar2b_in", name="ar2b_in")
        ar2b_out = dram.tile([1, 1], f32, tag="ar2b_out", name="ar2b_out")
        ar3_in = dram.tile([1, 1], f32, tag="ar3_in", name="ar3_in")
        ar3_out = dram.tile([1, 1], f32, tag="ar3_out", name="ar3_out")

        # PE warm-up burst first (flip HAM to 8/8)
        wut = pp.tile([128, 512], bf16, tag="wut", name="wut")
        nc.vector.memset(wut[:], 1.0)
        with tc.tile_pool(name="pswu", bufs=1, space="PSUM") as pswu:
            wps = pswu.tile([128, 512], f32, tag="wps", name="wps")
            for _ in range(20):
                nc.tensor.matmul(wps[:], lhsT=wut[:, 0:128], rhs=wut[:], start=True, stop=True)
        nc.vector.memset(magic_col[:], MAGIC)
        for h in range(H):
            nc.gpsimd.memset(aug_q[h][64:66, :], 0.0)
            nc.gpsimd.memset(aug_k[h][64:66, :], 1.0)
        with tc.tile_pool(name="brow", bufs=1) as br:
            bvrow = br.tile([1, C], f32, tag="bvrow", name="bvrow")
            nc.sync.dma_start(bvrow[:], bqs_ext[0:1, 1536:2304])
            nc.gpsimd.partition_broadcast(bv_bc[:], bvrow[:])

        # late pool: tensors born mid-kernel (vz at v-quant, bp_bc at proj)
        with tc.tile_pool(name="late", bufs=1) as lp:
         # padded PV weights, persistent: vz[hp][:, jt*256 + 0:64] = v cols of
         # head 2hp (key-block jt), [jt*256+192:256] = head 2hp+1, gaps zero.
         vz = [lp.tile([128, 8 * 256], bf16, tag=f"vz{t}", name=f"vz{t}") for t in range(6)]
         bp_bc = lp.tile([128, C], f32, tag="bp_bc", name="bp_bc")
         # v_f persists until v-quantize (after AR#2a)
         with tc.tile_pool(name="vf_pool", bufs=1) as vfp:
          v_f = [vfp.tile([128, C], f32, tag=f"vf{t}", name=f"vf{t}") for t in range(8)]

          # ---- stage 1+2: x quant, QKV matmuls, AR#1 (q,k), quantize -----
          with (
            tc.tile_pool(name="wload", bufs=1) as wl,
            tc.tile_pool(name="qkvf_pool", bufs=1) as qp,
            tc.tile_pool(name="s12", bufs=2) as s12,
            tc.tile_pool(name="psq", bufs=4, space="PSUM") as psq,
          ):
            wq_bf = [wl.tile([128, 3 * C], bf16, tag=f"wq{t}", name=f"wq{t}") for t in range(6)]
            xq = [wl.tile([128, N], bf16, tag=f"xq{t}", name=f"xq{t}") for t in range(6)]
            qkv_f = [qp.tile([128, N], f32, tag=f"qkvf{t}", name=f"qkvf{t}") for t in range(12)]

            for t in range(6):
                xf = s12.tile([128, N], f32, tag="s12y", name="xf")
                nc.sync.dma_start(xf[:], xT_ext[t * 128 : (t + 1) * 128, :])
                nc.scalar.dma_start(wq_bf[t][:], wq_qkv_ext[t * 128 : (t + 1) * 128, :])
                # x quantize: scale+magic on Act (idle in this window), round on DVE
                y = s12.tile([128, N], f32, tag="s12y", name="y")
                nc.scalar.activation(y[:], xf[:], ACT.Identity, bias=magic_col[:], scale=inv_s_x)
                nc.vector.tensor_scalar(xq[t][:], y[:], MAGIC, None, ALU.subtract)
            for mt in range(12):
                nc.sync.dma_start(
                    bqs_cols[:, mt : mt + 1],
                    bqs_ext[0:1, mt * 128 : (mt + 1) * 128].rearrange("a (p b) -> (a p) b", b=1),
                )

            # q/k part: per-mt accumulation over kt
            for mt in range(12):
                pss = [psq.tile([128, 512], f32, tag="psq", name="psq", bufs=6) for _ in range(2)]
                for kt in range(6):
                    for nk in range(2):
                        mi = nc.tensor.matmul(
                            pss[nk][:],
                            lhsT=wq_bf[kt][:, mt * 128 : (mt + 1) * 128],
                            rhs=xq[kt][:, nk * 512 : (nk + 1) * 512],
                            start=(kt == 0),
                            stop=(kt == 5),
                            skip_group_check=True,
                        )
                        if nk == 1:
                            mi.ins.ldweights = False
                for nk in range(2):
                    # bias-add + PSUM->SBUF on Act; absmax post-bias on DVE
                    nc.scalar.activation(
                        qkv_f[mt][:, nk * 512 : (nk + 1) * 512],
                        pss[nk][:],
                        ACT.Identity,
                        bias=bqs_cols[:, mt : mt + 1],
                    )
                    nc.vector.tensor_reduce(
                        qkv_abs[:, mt * 2 + nk : mt * 2 + nk + 1],
                        qkv_f[mt][:, nk * 512 : (nk + 1) * 512],
                        axis=AX.X,
                        op=ALU.max,
                        apply_absolute_value=True,
                    )

            # ---- AR#1: global absmax of q, k only --------------------------
            # (absmax computed pre-bias on PSUM would be wrong; bias shifts
            # values. Reduce from qkv_f instead -- see note below.)
            am2 = s12.tile([128, 2], f32, tag="am2", name="am2")
            nc.vector.tensor_reduce(am2[:, 0:1], qkv_abs[:, 0:12], axis=AX.X, op=ALU.max)
            nc.vector.tensor_reduce(am2[:, 1:2], qkv_abs[:, 12:24], axis=AX.X, op=ALU.max)
            am2r = s12.tile([128, 2], f32, tag="am2r", name="am2r")
            nc.gpsimd.partition_all_reduce(am2r[:], am2[:], 128, RED.max)
            nc.gpsimd.dma_start(ar1_in[:], am2r[0:1, :])
            nc.gpsimd.collective_compute(
                "AllReduce", ALU.max, replica_groups=RG, ins=[ar1_in.opt()], outs=[ar1_out.opt()]
            )
            # keep-warm during AR#1 (runs alongside the v matmuls below)
            _warm_burst(nc, psq, s12, wut, am2r, 40, 1, tag="psq", shape=[128, 512], bufs=6)
            g2 = pp.tile([1, 2], f32, tag="g2", name="g2")
            nc.gpsimd.dma_start(g2[:], ar1_out[:])
            g2b = pp.tile([128, 2], f32, tag="g2b", name="g2b")
            nc.gpsimd.partition_broadcast(g2b[:], g2[:])

            inv2 = pp.tile([128, 2], f32, tag="inv2", name="inv2")
            nc.vector.reciprocal(inv2[:], g2b[:])
            nc.vector.tensor_scalar(inv2[:], inv2[:], 127.0, None, ALU.mult)
            nc.vector.tensor_tensor(sc[:, 3:4], g2b[:, 0:1], g2b[:, 1:2], ALU.mult)
            nc.vector.tensor_scalar(sc[:, 3:4], sc[:, 3:4], CATT_CONST[0], None, ALU.mult)
            nc.vector.reciprocal(sc[:, 9:10], sc[:, 3:4])
            # v matmuls fill the PE idle window during/after AR#1
            for nt in range(8):
                pss = []
                for ick, (ck, cw) in enumerate(((0, 512), (512, 256))):
                    pss.append((psq.tile([128, 512], f32, tag="psq", name="psv", bufs=6), ck, cw))
                for kt in range(6):
                    for ick2, (ps, ck, cw) in enumerate(pss):
                        mi = nc.tensor.matmul(
                            ps[:, 0:cw],
                            lhsT=xq[kt][:, nt * 128 : (nt + 1) * 128],
                            rhs=wq_bf[kt][:, 1536 + ck : 1536 + ck + cw],
                            start=(kt == 0),
                            stop=(kt == 5),
                            skip_group_check=True,
                        )
                        if ick2 == 1:
                            mi.ins.ldweights = False
                for ick, (ps, ck, cw) in enumerate(pss):
                    nc.vector.scalar_tensor_tensor(
                        v_f[nt][:, ck : ck + cw],
                        ps[:, 0:cw],
                        1.0,
                        bv_bc[:, ck : ck + cw],
                        ALU.mult,
                        ALU.add,
                    )
                    nc.vector.tensor_reduce(
                        v_abs[:, nt * 2 + ick : nt * 2 + ick + 1],
                        v_f[nt][:, ck : ck + cw],
                        axis=AX.X,
                        op=ALU.max,
                        apply_absolute_value=True,
                    )

            # ---- AR#2a: global absmax of v, issued EARLY (hides under
            # phase A; result needed only at v-quantize) -------------------
            vam = s12.tile([128, 1], f32, tag="vam", name="vam")
            nc.vector.tensor_reduce(vam[:], v_abs[:], axis=AX.X, op=ALU.max)
            vamr = s12.tile([128, 1], f32, tag="vamr", name="vamr")
            nc.gpsimd.partition_all_reduce(vamr[:], vam[:], 128, RED.max)
            nc.gpsimd.dma_start(ar2a_in[:], vamr[0:1, :])
            nc.gpsimd.collective_compute(
                "AllReduce", ALU.max, replica_groups=RG, ins=[ar2a_in.opt()], outs=[ar2a_out.opt()]
            )
            gmv = pp.tile([1, 1], f32, tag="gmv", name="gmv")
            nc.gpsimd.dma_start(gmv[:], ar2a_out[:])
            nc.gpsimd.partition_broadcast(sc[:, 14:15], gmv[:])
            nc.vector.reciprocal(sc[:, 15:16], sc[:, 14:15])
            nc.vector.tensor_scalar(sc[:, 15:16], sc[:, 15:16], 127.0, None, ALU.mult)

            # ---- quantize q/k; build aug tensors ---------------------------
            # op1 (scale+magic, AP scalar) on DVE; op2 (round, imm) on gpsimd
            for mt in (0, 6, 1, 7, 2, 8, 3, 9, 4, 10, 5, 11):
                inv = inv2[:, 0:1] if mt < 6 else inv2[:, 1:2]
                y = s12.tile([128, N], f32, tag="s12y", name="yq")
                nc.vector.tensor_scalar(y[:], qkv_f[mt][:], inv, MAGIC, ALU.mult, ALU.add)
                qsc = s12.tile([128, N], bf16, tag="qsc", name="qsc")
                nc.vector.tensor_scalar(qsc[:], y[:], MAGIC, None, ALU.subtract)
                dst = aug_q if mt < 6 else aug_k
                tt = mt if mt < 6 else mt - 6
                nc.sync.dma_start(dst[2 * tt][0:64, :], qsc[0:64, :])
                nc.sync.dma_start(dst[2 * tt + 1][0:64, :], qsc[64:128, :])

            # zero-fill vz gaps on gpsimd (idle until phase B; must precede
            # the strided v-round writes at AR#2b)
            for t in range(6):
                nc.gpsimd.memset(vz[t][:], 0.0)

          # ---- phase A: attn logits stats (z via Act accum, rowmax of ea
          # via gpsimd max-tree + small DVE reduce) ------------------------
          with (
            tc.tile_pool(name="phA", bufs=4) as pa,
            tc.tile_pool(name="psA", bufs=3, space="PSUM") as psa,
          ):
            for h in range(H):
                for itp in range(4):
                    # paired it-blocks: one [128, 2N] f32 ea tile, one DVE
                    # rowmax. ea must stay f32: any rounding of the maxp path
                    # shifts the prob-quant grid and flips ~delta*21000 ints
                    # per row.
                    ea = pa.tile([128, 2 * N], f32, tag="ea", name="ea")
                    for s in range(2):
                        it = 2 * itp + s
                        psl = psa.tile([128, N], f32, tag="psl", name="psl")
                        for jc in range(2):
                            mi = nc.tensor.matmul(
                                psl[:, jc * 512 : (jc + 1) * 512],
                                lhsT=aug_q[h][0:66, it * 128 : (it + 1) * 128],
                                rhs=aug_k[h][0:66, jc * 512 : (jc + 1) * 512],
                                start=True,
                                stop=True,
                            )
                            if jc == 1:
                                mi.ins.ldweights = False
                        col = h * 8 + it
                        nc.scalar.activation(
                            ea[:, s * N : (s + 1) * N], psl[:], ACT.Exp, scale=sc[:, 3:4],
                            accum_out=zbuf[:, col : col + 1],
                        )
                    # rowmax(ea) == exp(scale*rowmax(logits)); maxp = rowmax*rz
                    col = h * 8 + 2 * itp
                    nc.vector.tensor_reduce(
                        mlbuf[:, col : col + 2],
                        ea[:].rearrange("p (s n) -> p s n", s=2),
                        axis=AX.X,
                        op=ALU.max,
                    )

          # ---- AR#2b: global max prob; ln(r)/c rows ----------------------
          with (
            tc.tile_pool(name="phR", bufs=1) as pr,
            tc.tile_pool(name="psT", bufs=1, space="PSUM") as pst,
          ):
            from concourse.masks import make_identity

            rz = pr.tile([128, 96], f32, tag="rz", name="rz")
            nc.vector.reciprocal(rz[:], zbuf[:])
            mp = pr.tile([128, 96], f32, tag="mp", name="mp")
            nc.vector.tensor_tensor(mp[:], mlbuf[:], rz[:], ALU.mult)
            pk2 = pr.tile([128, 1], f32, tag="pk2", name="pk2")
            nc.vector.tensor_reduce(pk2[:], mp[:], axis=AX.X, op=ALU.max)
            pk2r = pr.tile([128, 1], f32, tag="pk2r", name="pk2r")
            nc.gpsimd.partition_all_reduce(pk2r[:], pk2[:], 128, RED.max)
            nc.gpsimd.dma_start(ar2b_in[:], pk2r[0:1, :])
            nc.gpsimd.collective_compute(
                "AllReduce", ALU.max, replica_groups=RG, ins=[ar2b_in.opt()], outs=[ar2b_out.opt()]
            )
            gmp = pr.tile([1, 1], f32, tag="gmp", name="gmp")
            nc.gpsimd.dma_start(gmp[:], ar2b_out[:])
            nc.gpsimd.partition_broadcast(sc[:, 7:8], gmp[:])

            # v quantize fills the AR#2b latency window (needs only AR#2a):
            # scale+magic on gpsimd, strided round writes directly into vz
            for nt in range(8):
                y = pr.tile([128, C], f32, tag="yv", name="yv", bufs=2)
                nc.vector.tensor_scalar(y[:], v_f[nt][:], sc[:, 15:16], MAGIC, ALU.mult, ALU.add)
                for hp in range(6):
                    h0, h1 = 2 * hp, 2 * hp + 1
                    nc.vector.tensor_scalar(
                        vz[hp][:, nt * 256 : nt * 256 + 64],
                        y[:, h0 * 64 : (h0 + 1) * 64],
                        MAGIC, None, ALU.subtract,
                    )
                    nc.vector.tensor_scalar(
                        vz[hp][:, nt * 256 + 192 : nt * 256 + 256],
                        y[:, h1 * 64 : (h1 + 1) * 64],
                        MAGIC, None, ALU.subtract,
                    )

            # keep-warm during AR#2b
            _warm_burst(nc, pst, pr, wut, pk2r, 60, 2, tag="psT2", shape=[128, 512])

            # r rows (need AR#2b): r = 127*rz/maxp
            nc.vector.reciprocal(sc[:, 8:9], sc[:, 7:8])
            nc.vector.tensor_scalar(sc[:, 8:9], sc[:, 8:9], 127.0, None, ALU.mult)
            rbuf = pr.tile([128, 96], f32, tag="rbuf", name="rbuf")
            nc.vector.tensor_scalar(rbuf[:], rz[:], sc[:, 8:9], None, ALU.mult)
            lnr = pr.tile([128, 96], f32, tag="lnr", name="lnr")
            nc.scalar.activation(lnr[:], rbuf[:], ACT.Ln)
            lnrc = pr.tile([128, 128], f32, tag="lnrc", name="lnrc")
            nc.vector.memset(lnrc[:], 0.0)
            nc.vector.tensor_scalar(lnrc[:, 0:96], lnr[:], sc[:, 9:10], None, ALU.mult)

            idn = pr.tile([128, 128], f32, tag="idn", name="idn")
            make_identity(nc, idn[:])
            psT = pst.tile([128, 128], f32, tag="psT", name="psT")
            nc.tensor.transpose(psT[:], lnrc[:], idn[:])
            lnrcT = pr.tile([128, 128], f32, tag="lnrcT", name="lnrcT")
            nc.scalar.activation(lnrcT[:], psT[:], ACT.Copy)
            lnrc_rows = pr.tile([H, N], f32, tag="lnrc_rows", name="lnrc_rows")
            hi_rows = pr.tile([H, N], bf16, tag="hi_rows", name="hi_rows")
            lo_rows = pr.tile([H, N], bf16, tag="lo_rows", name="lo_rows")
            for h in range(H):
                nc.sync.dma_start(lnrc_rows[h : h + 1, :], lnrcT[h * 8 : (h + 1) * 8, :])
            nc.vector.tensor_copy(hi_rows[:], lnrc_rows[:])
            nc.vector.scalar_tensor_tensor(
                lo_rows[:], lnrc_rows[:], 1.0, hi_rows[:], ALU.mult, ALU.subtract
            )
            for h in range(H):
                nc.sync.dma_start(aug_q[h][64:65, :], hi_rows[h : h + 1, :])
                nc.sync.dma_start(aug_q[h][65:66, :], lo_rows[h : h + 1, :])

         # ---- phase B: quantized probs + PV (persistent padded vz) -------
         with tc.tile_pool(name="oint_pool", bufs=1) as op_:
           o_int = [op_.tile([128, N], f32, tag=f"oint{t}", name=f"oint{t}") for t in range(6)]
           wp_bf = [op_.tile([128, C], bf16, tag=f"wp{t}", name=f"wp{t}") for t in range(6)]
           for t in range(6):
               nc.sync.dma_start(wp_bf[t][:], wq_proj_ext[t * 128 : (t + 1) * 128, :])
           with (
             tc.tile_pool(name="phB", bufs=4) as pb,
             tc.tile_pool(name="psB", bufs=3, space="PSUM") as psb,
             tc.tile_pool(name="psO", bufs=1, space="PSUM") as pso_pool,
           ):
             for hp in range(6):
                 h0, h1 = 2 * hp, 2 * hp + 1
                 pso = pso_pool.tile([128, N], f32, tag="pso", name="pso")
                 for jt in range(8):
                     pqs = []
                     for hh, h in enumerate((h0, h1)):
                         pslT = psb.tile([128, N], f32, tag="pslT", name="pslT")
                         for ic in range(2):
                             mi = nc.tensor.matmul(
                                 pslT[:, ic * 512 : (ic + 1) * 512],
                                 lhsT=aug_k[h][0:66, jt * 128 : (jt + 1) * 128],
                                 rhs=aug_q[h][0:66, ic * 512 : (ic + 1) * 512],
                                 start=True,
                                 stop=True,
                             )
                             if ic == 1:
                                 mi.ins.ldweights = False
                         ep = pb.tile([128, N], f32, tag="ep", name="ep")
                         nc.scalar.activation(ep[:], pslT[:], ACT.Exp, scale=sc[:, 3:4])
                         pq = pb.tile([128, N], bf16, tag="pq", name="pq")
                         nc.vector.tensor_scalar(pq[:], ep[:], MAGIC, MAGIC, ALU.add, ALU.subtract)
                         pqs.append(pq)
                     for hh, pq in enumerate(pqs):
                         for ic in range(2):
                             mi = nc.tensor.matmul(
                                 pso[:, ic * 512 : (ic + 1) * 512],
                                 lhsT=vz[hp][:, jt * 256 + hh * 128 : jt * 256 + (hh + 1) * 128],
                                 rhs=pq[:, ic * 512 : (ic + 1) * 512],
                                 start=(jt == 0 and hh == 0),
                                 stop=(jt == 7 and hh == 1),
                                 skip_group_check=True,
                             )
                             if ic == 1:
                                 mi.ins.ldweights = False
                 nc.vector.tensor_copy(o_int[hp][:], pso[:])
                 nc.vector.tensor_reduce(
                     o_abs[:, hp : hp + 1], pso[:], axis=AX.X, op=ALU.max, apply_absolute_value=True
                 )

           # ---- AR#3 + quantize o + proj ----------------------------------
           with (
             tc.tile_pool(name="phC", bufs=3) as pc,
             tc.tile_pool(name="oq_pool", bufs=1) as oqp,
             tc.tile_pool(name="psF", bufs=4, space="PSUM") as psf_pool,
           ):
             bprow = pc.tile([1, C], f32, tag="bprow", name="bprow")
             nc.sync.dma_start(bprow[:], bp_ext[:])
             nc.gpsimd.partition_broadcast(bp_bc[:], bprow[:])
             oam = pc.tile([128, 1], f32, tag="oam", name="oam")
             nc.vector.tensor_reduce(oam[:], o_abs[:], axis=AX.X, op=ALU.max)
             oamr = pc.tile([128, 1], f32, tag="oamr", name="oamr")
             nc.gpsimd.partition_all_reduce(oamr[:], oam[:], 128, RED.max)
             nc.gpsimd.dma_start(ar3_in[:], oamr[0:1, :])
             nc.gpsimd.collective_compute(
                 "AllReduce", ALU.max, replica_groups=RG, ins=[ar3_in.opt()], outs=[ar3_out.opt()]
             )
             go = pc.tile([1, 1], f32, tag="go", name="go")
             nc.gpsimd.dma_start(go[:], ar3_out[:])
             nc.gpsimd.partition_broadcast(sc[:, 10:11], go[:])

             # keep-warm during AR#3
             _warm_burst(nc, psf_pool, pc, wut, oamr, 40, 3, tag="psf", shape=[128, C])

             nc.vector.reciprocal(sc[:, 11:12], sc[:, 10:11])
             nc.vector.tensor_scalar(sc[:, 11:12], sc[:, 11:12], 127.0, None, ALU.mult)
             nc.vector.tensor_tensor(sc[:, 12:13], sc[:, 7:8], sc[:, 14:15], ALU.mult)
             nc.vector.tensor_tensor(sc[:, 12:13], sc[:, 12:13], sc[:, 10:11], ALU.mult)
             nc.vector.tensor_scalar(sc[:, 12:13], sc[:, 12:13], SFIN_CONST[0], None, ALU.mult)

             oq = [oqp.tile([128, N], bf16, tag=f"oq{t}", name=f"oq{t}") for t in range(6)]
             for t in range(6):
                 y = pc.tile([128, N], f32, tag="yo", name="yo")
                 nc.scalar.activation(y[:], o_int[t][:], ACT.Identity, bias=magic_col[:], scale=sc[:, 11:12])
                 nc.vector.tensor_scalar(oq[t][:], y[:], MAGIC, None, ALU.subtract)

             for g in range(2):
                 psfs = [psf_pool.tile([128, C], f32, tag="psf", name="psf") for _ in range(4)]
                 for kt in range(6):
                     for nn in range(4):
                         nt = g * 4 + nn
                         for ick2, (ck, cw) in enumerate(((0, 512), (512, 256))):
                             mi = nc.tensor.matmul(
                                 psfs[nn][:, ck : ck + cw],
                                 lhsT=oq[kt][:, nt * 128 : (nt + 1) * 128],
                                 rhs=wp_bf[kt][:, ck : ck + cw],
                                 start=(kt == 0),
                                 stop=(kt == 5),
                                 skip_group_check=True,
                             )
                             if ick2 == 1:
                                 mi.ins.ldweights = False
                 for nn in range(4):
                     nt = g * 4 + nn
                     ot = pc.tile([128, C], f32, tag="ot", name="ot")
                     nc.vector.scalar_tensor_tensor(
                         ot[:], psfs[nn][:], sc[:, 12:13], bp_bc[:], ALU.mult, ALU.add
                     )
                     eng = nc.sync if nn % 2 == 0 else nc.scalar
                     eng.dma_start(out_ext[nt * 128 : (nt + 1) * 128, :], ot[:])


def _host_prep(x, w_qkv, b_qkv, w_proj, b_proj):
    x = np.asarray(x, dtype=np.float32)
    w_qkv = np.asarray(w_qkv, dtype=np.float32)
    b_qkv = np.asarray(b_qkv, dtype=np.float32)
    w_proj = np.asarray(w_proj, dtype=np.float32)
    b_proj = np.asarray(b_proj, dtype=np.float32)

    qmax = np.float32(127.0)
    s_x = np.maximum(np.max(np.abs(x)) / qmax, np.float32(1e-8))
    s_wq = np.maximum(np.max(np.abs(w_qkv)) / qmax, np.float32(1e-8))
    s_wp = np.maximum(np.max(np.abs(w_proj)) / qmax, np.float32(1e-8))
    inv_s_x = float(np.float32(1.0) / s_x)

    wq_qkv = np.round(w_qkv / s_wq).astype(ml_dtypes.bfloat16)
    wq_proj = np.round(w_proj / s_wp).astype(ml_dtypes.bfloat16)
    bqs = (b_qkv / (s_x * s_wq)).astype(np.float32)[None, :]
    bp = b_proj.astype(np.float32)[None, :]

    sxw = float(s_x) * float(s_wq)
    sfin = float(s_wp) * sxw / (127.0**3)
    catt = 0.125 * sxw * sxw / (127.0 * 127.0)
    in_maps = [
        {
            "xT": np.ascontiguousarray(x[b].T),
            "wq_qkv": wq_qkv,
            "wq_proj": wq_proj,
            "bqs": bqs,
            "bp": bp,
        }
        for b in range(B)
    ]
    return inv_s_x, sfin, catt, in_maps


_CACHE = {}


def kernel(x, w_qkv, b_qkv, w_proj, b_proj):
    inv_s_x, sfin, catt, in_maps = _host_prep(x, w_qkv, b_qkv, w_proj, b_proj)
    key = (inv_s_x, sfin, catt)
    if key not in _CACHE:
        SFIN_CONST[0] = sfin
        CATT_CONST[0] = catt
        _CACHE[key] = build_graph(inv_s_x)
    nc = _CACHE[key]
    res = run_bass_kernel_spmd(nc, in_maps, CORES)
    return np.stack([res.results[b]["out"] for b in range(B)], axis=0)


def build_and_inmaps(x, w_qkv, b_qkv, w_proj, b_proj):
    inv_s_x, sfin, catt, in_maps = _host_prep(x, w_qkv, b_qkv, w_proj, b_proj)
    SFIN_CONST[0] = sfin
    CATT_CONST[0] = catt
    nc = build_graph(inv_s_x)
    return nc, in_maps
